# revision 1
# baseline (speedup 1.0000x reference)
"""LSNN cell single-step kernel for Trainium2, data-parallel over 8 NeuronCores.

Full-input contract: kernel(**inputs) takes the unsharded tensors
(B=8192, IN_F=512, OUT_F=1024) and returns the stacked [4, B, OUT_F]
(z_new, v_new, i_new, b_new) fp32 output.

Sharding: batch 8192 -> 8 cores x 1024 rows. Weights are replicated,
host-transposed to [K, N] ("rhs") layout and cast to bf16 (the spike
matmul operands are exactly 0/1 so the only rounding is in the weights).
All threshold-feeding elementwise math is fp32 with the reference's
exact operation order so z_new/v_new/b_new are bit-exact.
"""

import sys
import types
from contextlib import ExitStack

import numpy as np
import ml_dtypes

# bass_utils imports antenv.axon_hooks when tracing is requested (e.g. via a
# BASS_TRACE env var); this image's antenv package lacks that module. Register
# a fallback shim that reports "no hook" so tracing degrades instead of
# crashing. test.py overwrites the getter with a real ctypes-backed hook.
if "antenv.axon_hooks" not in sys.modules:
    _shim = types.ModuleType("antenv.axon_hooks")
    _shim._hook = None
    _shim.get_axon_ntff_profile_hook = lambda: _shim._hook

    def _set_hook(h):
        _shim._hook = h

    _shim.set_axon_ntff_profile_hook = _set_hook
    import antenv  # noqa: F401  (make the parent package importable first)

    sys.modules["antenv.axon_hooks"] = _shim

import concourse.bass as bass
import concourse.tile as tile
from concourse import bacc, mybir
from concourse.bass_utils import run_bass_kernel_spmd
from concourse.masks import make_identity

F32 = mybir.dt.float32
BF16 = mybir.dt.bfloat16
ALU = mybir.AluOpType
ACT_COPY = mybir.ActivationFunctionType.Copy

N_CORES = 8
B, IN_F, OUT_F = 8192, 512, 1024
B_CORE = B // N_CORES          # 1024 rows per core
P = 128                        # partitions
KI = IN_F // P                 # 4 contraction chunks for the input matmul
KO = OUT_F // P                # 8 contraction chunks for the recurrent matmul

# Constants, replicating the reference's jax fp32 arithmetic exactly.
# python-double products are cast to fp32 once multiplied with fp32 arrays;
# instruction immediates are stored as fp32, giving the same cast.
C_VDEC = 0.001 * 100.0                   # DT * TAU_MEM_INV
C_BDEC = 0.001 * (1.0 / 800.0)           # DT * TAU_ADAPT_INV
C_IDEC = 0.001 * (-200.0)                # DT * (-TAU_SYN_INV)
# reference computes (z * f32(TAU_ADAPT_INV)) * f32(BETA); with z in {0,1}
# that's z * (f32(1/800) *f32 f32(1.8)) exactly.
C_BJUMP = float(np.float32(np.float32(1.0 / 800.0) * np.float32(1.8)))


def build_nc(n_btiles: int = B_CORE // P):
    """Emit the per-core Tile kernel for `n_btiles` batch tiles of 128."""
    rows = n_btiles * P
    nc = bacc.Bacc(
        "TRN2",
        target_bir_lowering=False,
        debug=False,
        enable_asserts=False,
        num_devices=N_CORES,
    )
    s_d = nc.dram_tensor("in_spikes", [rows, IN_F], F32, kind="ExternalInput").ap()
    z_d = nc.dram_tensor("in_z", [rows, OUT_F], F32, kind="ExternalInput").ap()
    v_d = nc.dram_tensor("in_v", [rows, OUT_F], F32, kind="ExternalInput").ap()
    i_d = nc.dram_tensor("in_i", [rows, OUT_F], F32, kind="ExternalInput").ap()
    b_d = nc.dram_tensor("in_b", [rows, OUT_F], F32, kind="ExternalInput").ap()
    wiT_d = nc.dram_tensor("in_wiT", [IN_F, OUT_F], BF16, kind="ExternalInput").ap()
    wrT_d = nc.dram_tensor("in_wrT", [OUT_F, OUT_F], BF16, kind="ExternalInput").ap()
    out_d = nc.dram_tensor("out", [4, rows, OUT_F], F32, kind="ExternalOutput").ap()

    with tile.TileContext(nc) as tc, ExitStack() as ctx:
        const_pool = ctx.enter_context(tc.tile_pool(name="const", bufs=1))
        w_pool = ctx.enter_context(tc.tile_pool(name="weights", bufs=1))
        in_pool = ctx.enter_context(tc.tile_pool(name="inp", bufs=3))
        lhsT_pool = ctx.enter_context(tc.tile_pool(name="lhsT", bufs=2))
        tmp_pool = ctx.enter_context(tc.tile_pool(name="tmp", bufs=2))
        out_pool = ctx.enter_context(tc.tile_pool(name="outp", bufs=2))
        psum_tr = ctx.enter_context(
            tc.tile_pool(name="psum_tr", bufs=4, space="PSUM")
        )
        psum_mm = ctx.enter_context(
            tc.tile_pool(name="psum_mm", bufs=2, space="PSUM")
        )

        ident = const_pool.tile([P, P], F32)
        make_identity(nc, ident)

        wiT = w_pool.tile([P, KI, OUT_F], BF16)
        nc.sync.dma_start(wiT, wiT_d.rearrange("(c p) n -> p c n", p=P))
        wrT = w_pool.tile([P, KO, OUT_F], BF16)
        nc.sync.dma_start(wrT, wrT_d.rearrange("(c p) n -> p c n", p=P))

        for t in range(n_btiles):
            rs = bass.ts(t, P)  # this tile's 128 batch rows

            s_t = in_pool.tile([P, IN_F], F32, tag="s")
            nc.sync.dma_start(s_t, s_d[rs, :])
            z_t = in_pool.tile([P, OUT_F], F32, tag="z")
            nc.sync.dma_start(z_t, z_d[rs, :])
            v_t = in_pool.tile([P, OUT_F], F32, tag="v")
            nc.sync.dma_start(v_t, v_d[rs, :])
            i_t = in_pool.tile([P, OUT_F], F32, tag="i")
            nc.sync.dma_start(i_t, i_d[rs, :])
            b_t = in_pool.tile([P, OUT_F], F32, tag="b")
            nc.sync.dma_start(b_t, b_d[rs, :])

            # Transpose the spike operands 128x128-blockwise on PE and cast
            # to bf16 on the way out of PSUM; these become matmul lhsT.
            sT = lhsT_pool.tile([P, KI, P], BF16, tag="sT")
            for k in range(KI):
                ps = psum_tr.tile([P, P], F32, tag="tr")
                nc.tensor.transpose(ps, s_t[:, bass.ts(k, P)], ident)
                nc.scalar.activation(sT[:, k, :], ps, ACT_COPY)
            zT = lhsT_pool.tile([P, KO, P], BF16, tag="zT")
            for k in range(KO):
                ps = psum_tr.tile([P, P], F32, tag="tr")
                nc.tensor.transpose(ps, z_t[:, bass.ts(k, P)], ident)
                nc.scalar.activation(zT[:, k, :], ps, ACT_COPY)

            # acc[:, j, :] accumulates spikes @ WiT + z @ WrT over 12 chunks.
            acc = psum_mm.tile([P, 2, OUT_F // 2], F32, tag="mm")
            for j in range(2):
                ns = bass.ts(j, OUT_F // 2)
                for k in range(KI):
                    nc.tensor.matmul(
                        acc[:, j, :], sT[:, k, :], wiT[:, k, ns],
                        start=(k == 0), stop=False,
                    )
                for k in range(KO):
                    nc.tensor.matmul(
                        acc[:, j, :], zT[:, k, :], wrT[:, k, ns],
                        start=False, stop=(k == KO - 1),
                    )

            # Elementwise chain (fp32, reference op order where it matters).
            t1 = tmp_pool.tile([P, OUT_F], F32, tag="t1")
            nc.vector.tensor_tensor(t1, i_t, v_t, ALU.subtract)  # i - v
            vdec = tmp_pool.tile([P, OUT_F], F32, tag="vdec")
            nc.vector.scalar_tensor_tensor(vdec, t1, C_VDEC, v_t, ALU.mult, ALU.add)
            one_m_b = tmp_pool.tile([P, OUT_F], F32, tag="omb")
            nc.scalar.activation(one_m_b, b_t, ACT_COPY, bias=1.0, scale=-1.0)
            bdec = tmp_pool.tile([P, OUT_F], F32, tag="bdec")
            nc.vector.scalar_tensor_tensor(
                bdec, one_m_b, C_BDEC, b_t, ALU.mult, ALU.add
            )
            nz = tmp_pool.tile([P, OUT_F], F32, tag="nz")
            nc.vector.tensor_tensor(nz, vdec, bdec, ALU.is_le)  # 1 - z_new

            v_o = out_pool.tile([P, OUT_F], F32, tag="vo")
            nc.vector.tensor_tensor(v_o, vdec, nz, ALU.mult)
            z_o = out_pool.tile([P, OUT_F], F32, tag="zo")
            nc.scalar.activation(z_o, nz, ACT_COPY, bias=1.0, scale=-1.0)

            idec = tmp_pool.tile([P, OUT_F], F32, tag="idec")
            nc.vector.scalar_tensor_tensor(idec, i_t, C_IDEC, i_t, ALU.mult, ALU.add)
            i_o = out_pool.tile([P, OUT_F], F32, tag="io")
            for j in range(2):
                ns = bass.ts(j, OUT_F // 2)
                nc.vector.tensor_tensor(i_o[:, ns], idec[:, ns], acc[:, j, :], ALU.add)

            b_o = out_pool.tile([P, OUT_F], F32, tag="bo")
            nc.vector.scalar_tensor_tensor(b_o, z_o, C_BJUMP, bdec, ALU.mult, ALU.add)

            nc.sync.dma_start(out_d[0, rs, :], z_o)
            nc.sync.dma_start(out_d[1, rs, :], v_o)
            nc.sync.dma_start(out_d[2, rs, :], i_o)
            nc.sync.dma_start(out_d[3, rs, :], b_o)

    nc.compile()
    return nc


_NC_CACHE = {}


def _get_nc(n_btiles: int = B_CORE // P):
    if n_btiles not in _NC_CACHE:
        _NC_CACHE[n_btiles] = build_nc(n_btiles)
    return _NC_CACHE[n_btiles]


def make_in_maps(input_spikes, z, v, i, b, input_weights, recurrent_weights):
    """Shard full inputs into per-core in_maps (batch split, weights repl)."""
    wiT = np.ascontiguousarray(
        np.asarray(input_weights, dtype=np.float32).T
    ).astype(ml_dtypes.bfloat16)
    wrT = np.ascontiguousarray(
        np.asarray(recurrent_weights, dtype=np.float32).T
    ).astype(ml_dtypes.bfloat16)
    maps = []
    for c in range(N_CORES):
        sl = slice(c * B_CORE, (c + 1) * B_CORE)
        maps.append(
            {
                "in_spikes": np.ascontiguousarray(input_spikes[sl], np.float32),
                "in_z": np.ascontiguousarray(z[sl], np.float32),
                "in_v": np.ascontiguousarray(v[sl], np.float32),
                "in_i": np.ascontiguousarray(i[sl], np.float32),
                "in_b": np.ascontiguousarray(b[sl], np.float32),
                "in_wiT": wiT,
                "in_wrT": wrT,
            }
        )
    return maps


def run_sharded(inputs: dict, trace: bool = False, **kw):
    """Compile (cached), run on 8 cores, return (full_output, raw_results)."""
    nc = _get_nc()
    in_maps = make_in_maps(**inputs)
    res = run_bass_kernel_spmd(
        nc, in_maps, list(range(N_CORES)), trace=trace, **kw
    )
    out = np.empty((4, B, OUT_F), dtype=np.float32)
    for c in range(N_CORES):
        out[:, c * B_CORE : (c + 1) * B_CORE, :] = res.results[c]["out"]
    return out, res


def kernel(**inputs) -> np.ndarray:
    out, _ = run_sharded(inputs, trace=False)
    return out
